# revision 25
# baseline (speedup 1.0000x reference)
"""Multi-head attention (N=2, K=2048, M=1024, H=16, D=64) on 8 TRN2 cores.

Sharding: tensor-parallel over heads — core c owns heads (2c, 2c+1).
Each core computes q/k/v projections for its 2 heads (full sequence),
attention, and a rank-128 partial of the output projection (its 128 rows
of Wo's input dim). Host sums the 8 partials and adds bo. No device
collectives.

On-device layouts (per core):
  xq/xk/xv [8 tb, 128 p, 8 mc, 512 f] bf16  host-tiled transposed inputs:
           [tb, p, mc, f] = x[tok=tb*512+f, m=mc*128+p], tok = n*2048+k
           -> one contiguous 1MB DMA per (tensor, tb)
  wq/wk/wv [1024 m, 128 hd] bf16   W[h,d,m] -> [m, hl*64+d] for local heads
  wo       [128 hd, 1024 mo] bf16  Wo[:, c*128:(c+1)*128].T
  bqk      [128, 2] f32, bv [128, 1] f32
  outT     [8 tb, 128 p, 8 mb, 512 f] bf16  tiled partial (1MB DMA per tb)

Attention is ONE continuous software pipeline over 128 iterations
(2 batches x 4 kq-quarters x 16 l-blocks) with no barriers: per
iteration the two heads' K=64 score matmuls run CONCURRENTLY on
disjoint PE row-groups (base partitions 0/64 -> tile_position row
packing), one [128,1024] ACTIVATE exps both heads, and AV matmuls
(ones-column trick, M=65) lag 12 iterations behind. Quarter
normalization (reciprocal_approx_fast + gpsimd partition_broadcast +
PSUM-direct multiply) and output-projection stripes are emitted
mid-stream, so ScalarE (the exp wall, ~143us) never starves at
quarter/batch boundaries. Projections are single-matmul fill steps
paced 2-4 per iteration into the PE's slack under ScalarE; a
deadline `require()` forces emission of any unit a consumer needs
(emission order defines Tile dependencies). Output partials are cast
to bf16 (halves the out-DMA); the host sums partials in f32.
"""
from collections import deque

import numpy as np
import ml_dtypes

import concourse.bass as bass
import concourse.tile as tile
from concourse.masks import make_identity
from concourse import bacc, mybir
from concourse.bass_utils import run_bass_kernel_spmd

F32 = mybir.dt.float32
BF16 = mybir.dt.bfloat16
BFNP = ml_dtypes.bfloat16

N_CORES = 8
DM = 1024          # d_model
TOK = 4096         # N*K tokens
SEQ = 2048         # tokens per batch
NB = 2             # batches
HC = 2             # heads per core
D = 64             # head dim

AV_LAG = 12        # iterations AV trails scores/exp
AV_LAG_TAIL = 3    # shrink lag near the end to cut the drain tail
DMA_VT = True      # v-transposes via DMA xbar instead of PE+DVE

_prog_cache = {}


class FillSched:
    """Named-unit fill scheduler. Units are atomic (they share the mm_ps
    pool and must not interleave with each other); steps within the
    active unit are paced out by PE cost (matmul steps cost 1, DVE/DMA
    steps cost 0). require(name) forces full emission of every unit up
    to and including `name` — emission order defines Tile dependencies,
    so any unit a consumer reads from MUST be emitted (not just queued)
    before the consumer."""

    def __init__(self):
        self.order = deque()      # (name, deque((fn, cost)))
        self.cur_name = None
        self.cur = deque()
        self.done = set()

    def add(self, name, unit):
        self.order.append((name, deque(unit)))

    def add_front(self, name, unit):
        self.order.appendleft((name, deque(unit)))

    def _finish_cur(self):
        while self.cur:
            self.cur.popleft()[0]()
        if self.cur_name is not None:
            self.done.add(self.cur_name)
            self.cur_name = None

    def pop_steps(self, budget):
        while budget > 0:
            if not self.cur:
                if self.cur_name is not None:
                    self.done.add(self.cur_name)
                    self.cur_name = None
                if not self.order:
                    return
                self.cur_name, self.cur = self.order.popleft()
            fn, cost = self.cur.popleft()
            fn()
            budget -= cost
        if not self.cur and self.cur_name is not None:
            self.done.add(self.cur_name)
            self.cur_name = None

    def require(self, name):
        if name in self.done:
            return
        if self.cur_name is not None:
            if self.cur_name == name:
                self._finish_cur()
                return
            self._finish_cur()
        while name not in self.done:
            assert self.order, f"unit {name} was never queued"
            self.cur_name, self.cur = self.order.popleft()
            self._finish_cur()

    def drain(self):
        self._finish_cur()
        while self.order:
            self.cur_name, self.cur = self.order.popleft()
            self._finish_cur()


def build_program():
    nc = bacc.Bacc("TRN2", target_bir_lowering=False, debug=False)

    xq = nc.dram_tensor("xq", [8, 128, 8, 512], BF16, kind="ExternalInput")
    xk = nc.dram_tensor("xk", [8, 128, 8, 512], BF16, kind="ExternalInput")
    xv = nc.dram_tensor("xv", [8, 128, 8, 512], BF16, kind="ExternalInput")
    wq = nc.dram_tensor("wq", [DM, 128], BF16, kind="ExternalInput")
    wk = nc.dram_tensor("wk", [DM, 128], BF16, kind="ExternalInput")
    wv = nc.dram_tensor("wv", [DM, 128], BF16, kind="ExternalInput")
    wo = nc.dram_tensor("wo", [128, DM], BF16, kind="ExternalInput")
    bqk = nc.dram_tensor("bqk", [128, 2], F32, kind="ExternalInput")
    bv = nc.dram_tensor("bv", [128, 1], F32, kind="ExternalInput")
    outT = nc.dram_tensor("outT", [8, 128, 8, 512], BF16, kind="ExternalOutput")

    Exp = mybir.ActivationFunctionType.Exp

    with tile.TileContext(nc) as tc:
        with (
            tc.tile_pool(name="const", bufs=1) as const,
            tc.tile_pool(name="big", bufs=1) as big,
            tc.tile_pool(name="xpool", bufs=1) as xpool,
            tc.tile_pool(name="attn", bufs=AV_LAG + 2) as attnp,
            tc.tile_pool(name="norm", bufs=2) as normp,
            tc.tile_pool(name="osb", bufs=2) as osb,
            tc.tile_pool(name="vtpool", bufs=3) as vtpool,
            tc.tile_pool(name="mm_ps", bufs=2, space="PSUM") as mm_ps,
            tc.tile_pool(name="sc_ps", bufs=2, space="PSUM") as sc_ps,
            tc.tile_pool(name="y_ps", bufs=2, space="PSUM") as y_ps,
        ):
            # ---- weights / biases ----
            wk_sb = const.tile([128, 8, 128], BF16, tag="wk")
            nc.sync.dma_start(wk_sb[:], wk.ap().rearrange("(c p) d -> p c d", p=128))
            wq_sb = const.tile([128, 8, 128], BF16, tag="wq")
            nc.sync.dma_start(wq_sb[:], wq.ap().rearrange("(c p) d -> p c d", p=128))
            wv_sb = const.tile([128, 8, 128], BF16, tag="wv")
            nc.sync.dma_start(wv_sb[:], wv.ap().rearrange("(c p) d -> p c d", p=128))
            wo_sb = const.tile([128, DM], BF16, tag="wo")
            nc.sync.dma_start(wo_sb[:], wo[:, :])
            bqk_sb = const.tile([128, 2], F32, tag="bqk")
            nc.sync.dma_start(bqk_sb[:], bqk[:, :])
            bv_sb = const.tile([128, 1], F32, tag="bv")
            nc.sync.dma_start(bv_sb[:], bv[:, :])
            # per-head v bias at partitions 0:64 (col h = head h)
            bv2_sb = const.tile([64, 2], F32, tag="bv2")
            nc.sync.dma_start(
                bv2_sb[:], bv.ap().rearrange("(h p) c -> p (h c)", h=2))
            # two stacked 64x64 identities so transposes of vT slices at
            # partition offsets 0 and 64 both have a matching-base identity
            ident = const.tile([128, 64], BF16, tag="ident")
            make_identity(nc, ident[0:64, :])
            nc.sync.dma_start(ident[64:128, :], ident[0:64, :])

            # ---- persistent activations ----
            qT = big.tile([128, TOK], BF16, tag="qT")     # [hd, tok]
            kT = big.tile([128, TOK], BF16, tag="kT")     # [hd, tok]
            # v blocks: 32 token-blocks of [128 tok, 2*(64+1)]; col 64 of each
            # per-head group is the ones column (softmax denominator trick)
            vA = big.tile([128, 32 * 130], BF16, tag="vA")
            yT = big.tile([128, TOK], BF16, tag="yT")     # attn out [hd, tok]

            nc.vector.memset(
                vA[:].rearrange("p (b h c) -> p b h c", h=2, c=65)[:, :, :, 64:65], 1.0
            )

            prefetched = {}

            def prefetch(key, tb):
                dram = {"q": xq, "k": xk, "v": xv}[key]
                # per-slot tag (bufs=1): batch-1's (key, tb+4) reuses exactly
                # the buffer of (key, tb), with a WAR dep on its reads
                t = xpool.tile([128, 8, 512], BF16,
                               tag=f"x{key}{tb % 4}", name="xt")
                nc.sync.dma_start(t[:], dram[tb])
                prefetched[(key, tb)] = t

            def proj_qk_steps(tb, which):
                """One qk projection as 8 single-MM closures (last one evicts)."""
                key, w_sb, dstT, bcol = (
                    ("q", wq_sb, qT, 0),
                    ("k", wk_sb, kT, 1),
                )[which]
                state = {}

                def step(mc):
                    if mc == 0:
                        state["xt"] = prefetched.pop((key, tb))
                        state["ps"] = mm_ps.tile([128, 512], F32, tag="mm", name="ps")
                    nc.tensor.matmul(
                        state["ps"][:], lhsT=w_sb[:, mc, :],
                        rhs=state["xt"][:, mc, :],
                        start=(mc == 0), stop=(mc == 7),
                    )
                    if mc == 7:
                        nc.vector.tensor_scalar_add(
                            dstT[:, tb * 512:(tb + 1) * 512], state["ps"][:],
                            bqk_sb[:, bcol:bcol + 1],
                        )
                        if tb < 4:
                            # batch-1 prefetch reuses this tile's buffer; it
                            # must be EMITTED after the last read of the old
                            # tile or the DMA races the projection
                            prefetch(key, tb + 4)
                return [(lambda mc=mc: step(mc), 1) for mc in range(8)]

            def proj_v_steps(tb):
                """One v projection: 8 single-MM closures, 2 per-head bias
                evicts, then 8 transpose+copy closures filling vA (DMA-xbar
                transpose into a zero-offset temp when DMA_VT)."""
                state = {}

                def step(mc):
                    if mc == 0:
                        state["xt"] = prefetched.pop(("v", tb))
                        state["ps"] = mm_ps.tile([128, 512], F32, tag="mm", name="ps")
                    nc.tensor.matmul(
                        state["ps"][:], lhsT=wv_sb[:, mc, :],
                        rhs=state["xt"][:, mc, :],
                        start=(mc == 0), stop=(mc == 7),
                    )
                    if mc == 7 and not DMA_VT:
                        state["vt"] = vtpool.tile(
                            [128, 512], BF16, tag="vt", name="vt")
                        nc.vector.tensor_scalar_add(
                            state["vt"][:], state["ps"][:], bv_sb[:, 0:1])
                    if mc == 7 and tb < 4:
                        prefetch("v", tb + 4)

                def evict(hl):
                    # per-head partition-0 tiles (xbar src must start at p0)
                    t = vtpool.tile([64, 512], BF16, tag=f"vt{hl}", name="vt")
                    nc.vector.tensor_scalar_add(
                        t[:], state["ps"][hl * 64:(hl + 1) * 64, :],
                        bv2_sb[:, hl:hl + 1])
                    state[f"vt{hl}"] = t

                def tstep(j, hl):
                    base = (tb * 4 + j) * 130
                    if DMA_VT:
                        tp = vtpool.tile([128, 64], BF16, tag="tp", name="tp")
                        nc.sync.dma_start(
                            tp[:], state[f"vt{hl}"][:, j * 128:(j + 1) * 128],
                            transpose=True)
                        # gpsimd (idle, off the norm critical path) scatters
                        # into the 65-stride vA layout
                        nc.gpsimd.tensor_copy(
                            vA[:, base + hl * 65: base + hl * 65 + 64], tp[:])
                    else:
                        tp = mm_ps.tile([128, 64], BF16, tag="mm", name="tp")
                        nc.tensor.transpose(
                            tp[:],
                            state["vt"][hl * 64:(hl + 1) * 64,
                                        j * 128:(j + 1) * 128],
                            ident[hl * 64:(hl + 1) * 64, :],
                        )
                        nc.vector.tensor_copy(
                            vA[:, base + hl * 65: base + hl * 65 + 64], tp[:])

                mm_steps = [(lambda mc=mc: step(mc), 1) for mc in range(8)]
                if DMA_VT:
                    return (mm_steps
                            + [(lambda hl=hl: evict(hl), 0) for hl in range(2)]
                            + [(lambda j=j, hl=hl: tstep(j, hl), 0)
                               for j in range(4) for hl in range(2)])
                return (mm_steps
                        + [(lambda j=j, hl=hl: tstep(j, hl), 1)
                           for j in range(4) for hl in range(2)])

            def out_proj_steps(n, qtr):
                """One 512-token output-projection stripe: 8 (MM + bf16 copy)
                closures; the last also DMAs the stripe out."""
                tb = n * 4 + qtr
                state = {}

                def step(mb):
                    if mb == 0:
                        state["o"] = osb.tile(
                            [128, 8, 512], BF16, tag="o", name="o_sb")
                    ps = mm_ps.tile([128, 512], F32, tag="mm", name="ps")
                    nc.tensor.matmul(
                        ps[:], lhsT=wo_sb[:, mb * 128:(mb + 1) * 128],
                        rhs=yT[:, tb * 512:(tb + 1) * 512],
                        start=True, stop=True,
                    )
                    nc.vector.tensor_copy(state["o"][:, mb, :], ps[:])
                    if mb == 7:
                        nc.sync.dma_start(outT[tb], state["o"][:])
                return [(lambda mb=mb: step(mb), 1) for mb in range(8)]

            fill = FillSched()

            def norm_qtr(n, qtr, yaccs):
                kq0 = n * SEQ + qtr * 512
                for h in range(2):
                    hp = h * 64
                    # single copy releases the yacc PSUM buffer fast; the
                    # rest of the chain runs off the PE critical path.
                    # (gpsimd can't access PSUM; custom DVE ops can't either)
                    ycp = normp.tile([65, 512], F32, tag="ycp", name="ycp")
                    nc.vector.tensor_copy(ycp[:], yaccs[h][:])
                    # custom DVE op needs a partition-0 input
                    dsb = normp.tile([1, 512], F32, tag="dsb", name="dsb")
                    nc.vector.tensor_copy(dsb[:], ycp[64:65, :])
                    recip = normp.tile([1, 512], F32, tag="recip", name="recip")
                    nc.vector.reciprocal_approx_fast(recip[:], dsb[:])
                    bcast = normp.tile([64, 512], F32, tag="bcast", name="bcast")
                    nc.gpsimd.partition_broadcast(bcast[:], recip[:])
                    nc.vector.tensor_mul(
                        yT[hp:hp + 64, kq0:kq0 + 512],
                        ycp[0:64, :], bcast[:],
                    )
                fill.add_front(f"out{n}{qtr}", out_proj_steps(n, qtr))

            def do_av(at, n, qtr, lb, yaccs):
                lt = n * 16 + lb
                fill.require(f"v{n * 4 + lb // 4}")
                for h in range(2):
                    nc.tensor.matmul(
                        yaccs[h][:],
                        lhsT=vA[:, lt * 130 + h * 65: lt * 130 + h * 65 + 65],
                        rhs=at[:, h * 512:(h + 1) * 512],
                        start=(lb == 0), stop=(lb == 15),
                    )
                if lb == 15:
                    norm_qtr(n, qtr, yaccs)

            # ---- prefetch: batch 0, k0/q0 in small chunks first so the
            # first projection matmuls start as soon as 128KB lands ----
            def prefetch_chunked(key, tb):
                dram = {"q": xq, "k": xk, "v": xv}[key]
                t = xpool.tile([128, 8, 512], BF16,
                               tag=f"x{key}{tb % 4}", name="xt")
                for mc in range(8):
                    nc.sync.dma_start(t[:, mc, :], dram[tb][:, mc, :])
                prefetched[(key, tb)] = t

            prefetch_chunked("k", 0)
            prefetch_chunked("q", 0)
            for tb in (1, 2, 3):
                prefetch("k", tb)
            for tb in range(4):
                prefetch("v", tb)
            for tb in (1, 2, 3):
                prefetch("q", tb)

            # ---- upfront: k0 + q0 projections only ----
            for f, _ in proj_qk_steps(0, 1):
                f()
            for f, _ in proj_qk_steps(0, 0):
                f()
            fill.done.update({"k0", "q0"})

            # deadline order: scores(lb) need k-tb(lb//4) / q-tb(qtr);
            # AV (lag 12) needs v-tb((lb-12)//4); batch 1 follows
            # (batch-1 prefetches are emitted by the consumption hooks
            # inside proj_*_steps — buffer-exact, race-free)
            units = [
                ("k1", proj_qk_steps(1, 1)), ("k2", proj_qk_steps(2, 1)),
                ("v0", proj_v_steps(0)), ("k3", proj_qk_steps(3, 1)),
                ("v1", proj_v_steps(1)), ("q1", proj_qk_steps(1, 0)),
                ("v2", proj_v_steps(2)), ("v3", proj_v_steps(3)),
                ("q2", proj_qk_steps(2, 0)), ("q3", proj_qk_steps(3, 0)),
                ("k4", proj_qk_steps(4, 1)), ("q4", proj_qk_steps(4, 0)),
                ("k5", proj_qk_steps(5, 1)), ("k6", proj_qk_steps(6, 1)),
                ("v4", proj_v_steps(4)), ("k7", proj_qk_steps(7, 1)),
                ("v5", proj_v_steps(5)), ("q5", proj_qk_steps(5, 0)),
                ("v6", proj_v_steps(6)), ("v7", proj_v_steps(7)),
                ("q6", proj_qk_steps(6, 0)), ("q7", proj_qk_steps(7, 0)),
            ]
            for name, unit in units:
                fill.add(name, unit)

            # ---- the continuous attention stream ----
            pend = deque()
            qtr_yaccs = {}
            NIT = NB * 64
            for i in range(NIT):
                n, r = divmod(i, 64)
                qtr, lb = divmod(r, 16)
                lt = n * 16 + lb
                kq0 = n * SEQ + qtr * 512
                if lb == 0:
                    qtr_yaccs[(n, qtr)] = [
                        y_ps.tile([65, 512], F32, tag="yacc", name="yacc")
                        for _ in range(2)
                    ]
                fill.require(f"k{n * 4 + lb // 4}")
                fill.require(f"q{n * 4 + qtr}")
                sp = sc_ps.tile([128, 1024], F32, tag="sc", name="sp")
                for h in range(2):
                    # K=64, base partitions 0/64 -> concurrent row-tiles
                    nc.tensor.matmul(
                        sp[:, h * 512:(h + 1) * 512],
                        lhsT=kT[h * 64:(h + 1) * 64, lt * 128:(lt + 1) * 128],
                        rhs=qT[h * 64:(h + 1) * 64, kq0:kq0 + 512],
                        start=True, stop=True,
                    )
                at = attnp.tile([128, 1024], BF16, tag="at", name="at")
                nc.scalar.activation(at[:], sp[:], Exp, scale=0.125)
                pend.append((at, n, qtr, lb, qtr_yaccs[(n, qtr)]))
                limit = AV_LAG if i < NIT - (AV_LAG - AV_LAG_TAIL) else AV_LAG_TAIL
                while len(pend) > limit:
                    args = pend.popleft()
                    do_av(*args)
                    if args[3] == 15:
                        del qtr_yaccs[(args[1], args[2])]
                fill.pop_steps(4 if i < 24 else (3 if i < 48 else 2))
            for args in pend:
                do_av(*args)
            fill.drain()

    nc.compile()
    return nc


def get_program():
    if "nc" not in _prog_cache:
        _prog_cache["nc"] = build_program()
    return _prog_cache["nc"]


def _tile_x(x):
    # [TOK, DM] f32 -> bf16 tiles [8 tb, 128 p, 8 mc, 512 f]:
    # t[tb,p,mc,f] = x[tb*512+f, mc*128+p]
    t = x.reshape(8, 512, 8, 128).astype(BFNP)   # [tb, f, mc, p]
    return np.ascontiguousarray(np.transpose(t, (0, 3, 2, 1)))


def make_in_maps(query, key, value, Wq, bq, Wk, bk, Wv, bv, Wo):
    """Host-side shard + layout. Returns list of 8 per-core input dicts."""
    xq = _tile_x(query.reshape(TOK, DM))
    xk = _tile_x(key.reshape(TOK, DM))
    xv = _tile_x(value.reshape(TOK, DM))

    in_maps = []
    for c in range(N_CORES):
        h0 = HC * c
        # W[h,d,m] slice -> [m, hl*64+d]
        wq_c = np.ascontiguousarray(
            np.transpose(Wq[h0:h0 + HC], (2, 0, 1)).reshape(DM, 128)).astype(BFNP)
        wk_c = np.ascontiguousarray(
            np.transpose(Wk[h0:h0 + HC], (2, 0, 1)).reshape(DM, 128)).astype(BFNP)
        wv_c = np.ascontiguousarray(
            np.transpose(Wv[h0:h0 + HC], (2, 0, 1)).reshape(DM, 128)).astype(BFNP)
        wo_c = np.ascontiguousarray(
            Wo[:, 128 * c:128 * (c + 1)].T).astype(BFNP)
        bqk_c = np.stack(
            [bq[h0:h0 + HC].reshape(128), bk[h0:h0 + HC].reshape(128)], axis=1
        ).astype(np.float32)
        bv_c = bv[h0:h0 + HC].reshape(128, 1).astype(np.float32)
        in_maps.append({
            "xq": xq, "xk": xk, "xv": xv,
            "wq": wq_c, "wk": wk_c, "wv": wv_c, "wo": wo_c,
            "bqk": bqk_c, "bv": bv_c,
        })
    return in_maps


def untile_out(res_list):
    """Sum per-core tiled bf16 partials in f32 -> [DM, TOK] f32."""
    acc = np.zeros((8, 128, 8, 512), np.float32)
    for r in res_list:
        acc += r["outT"].astype(np.float32)
    # [tb, p, mb, f] -> [mb*128+p, tb*512+f]
    return np.ascontiguousarray(np.transpose(acc, (2, 1, 0, 3))).reshape(DM, TOK)


def kernel(query, key, value, Wq, bq, Wk, bk, Wv, bv, Wo, bo):
    nc = get_program()
    in_maps = make_in_maps(query, key, value, Wq, bq, Wk, bk, Wv, bv, Wo)
    res = run_bass_kernel_spmd(nc, in_maps, list(range(N_CORES)))
    out_t = untile_out(res.results)
    out = out_t.T.reshape(NB, SEQ, DM) + bo.astype(np.float32)
    return out


# revision 26
# speedup vs baseline: 1.3767x; 1.3767x over previous
"""Multi-head attention (N=2, K=2048, M=1024, H=16, D=64) on 8 TRN2 cores.

Sharding: tensor-parallel over heads — core c owns heads (2c, 2c+1).
Each core computes q/k/v projections for its 2 heads (full sequence),
attention, and a rank-128 partial of the output projection (its 128 rows
of Wo's input dim). Host sums the 8 partials and adds bo. No device
collectives.

On-device layouts (per core):
  xq/xk/xv [8 tb, 128 p, 8 mc, 512 f] bf16  host-tiled transposed inputs:
           [tb, p, mc, f] = x[tok=tb*512+f, m=mc*128+p], tok = n*2048+k
           -> one contiguous 1MB DMA per (tensor, tb)
  wq/wk/wv [1024 m, 128 hd] bf16   W[h,d,m] -> [m, hl*64+d] for local heads
  wo       [128 hd, 1024 mo] bf16  Wo[:, c*128:(c+1)*128].T
  bqk      [128, 2] f32, bv [128, 1] f32
  outT     [8 tb, 128 p, 8 mb, 512 f] bf16  tiled partial (1MB DMA per tb)

Attention is ONE continuous software pipeline over 128 iterations
(2 batches x 4 kq-quarters x 16 l-blocks) with no barriers: per
iteration the two heads' K=64 score matmuls run CONCURRENTLY on
disjoint PE row-groups (base partitions 0/64 -> tile_position row
packing), one [128,1024] ACTIVATE exps both heads, and AV matmuls
(ones-column trick, M=65) lag 12 iterations behind. Quarter
normalization (reciprocal_approx_fast + gpsimd partition_broadcast +
PSUM-direct multiply) and output-projection stripes are emitted
mid-stream, so ScalarE (the exp wall, ~143us) never starves at
quarter/batch boundaries. Projections are single-matmul fill steps
paced 2-4 per iteration into the PE's slack under ScalarE; a
deadline `require()` forces emission of any unit a consumer needs
(emission order defines Tile dependencies). Output partials are cast
to bf16 (halves the out-DMA); the host sums partials in f32.
"""
from collections import deque

import numpy as np
import ml_dtypes

import concourse.bass as bass
import concourse.tile as tile
from concourse.masks import make_identity
from concourse import bacc, mybir
from concourse.bass_utils import run_bass_kernel_spmd

F32 = mybir.dt.float32
BF16 = mybir.dt.bfloat16
BFNP = ml_dtypes.bfloat16

N_CORES = 8
DM = 1024          # d_model
TOK = 4096         # N*K tokens
SEQ = 2048         # tokens per batch
NB = 2             # batches
HC = 2             # heads per core
D = 64             # head dim

AV_LAG = 12        # iterations AV trails scores/exp
AV_LAG_TAIL = 3    # shrink lag near the end to cut the drain tail
DMA_VT = False     # v-transposes via DMA xbar instead of PE+DVE

_prog_cache = {}


class FillSched:
    """Named-unit fill scheduler. Units are atomic (they share the mm_ps
    pool and must not interleave with each other); steps within the
    active unit are paced out by PE cost (matmul steps cost 1, DVE/DMA
    steps cost 0). require(name) forces full emission of every unit up
    to and including `name` — emission order defines Tile dependencies,
    so any unit a consumer reads from MUST be emitted (not just queued)
    before the consumer."""

    def __init__(self):
        self.order = deque()      # (name, deque((fn, cost)))
        self.cur_name = None
        self.cur = deque()
        self.done = set()

    def add(self, name, unit):
        self.order.append((name, deque(unit)))

    def add_front(self, name, unit):
        self.order.appendleft((name, deque(unit)))

    def _finish_cur(self):
        while self.cur:
            self.cur.popleft()[0]()
        if self.cur_name is not None:
            self.done.add(self.cur_name)
            self.cur_name = None

    def pop_steps(self, budget):
        while budget > 0:
            if not self.cur:
                if self.cur_name is not None:
                    self.done.add(self.cur_name)
                    self.cur_name = None
                if not self.order:
                    return
                self.cur_name, self.cur = self.order.popleft()
            fn, cost = self.cur.popleft()
            fn()
            budget -= cost
        if not self.cur and self.cur_name is not None:
            self.done.add(self.cur_name)
            self.cur_name = None

    def require(self, name):
        if name in self.done:
            return
        if self.cur_name is not None:
            if self.cur_name == name:
                self._finish_cur()
                return
            self._finish_cur()
        while name not in self.done:
            assert self.order, f"unit {name} was never queued"
            self.cur_name, self.cur = self.order.popleft()
            self._finish_cur()

    def drain(self):
        self._finish_cur()
        while self.order:
            self.cur_name, self.cur = self.order.popleft()
            self._finish_cur()


def build_program():
    nc = bacc.Bacc("TRN2", target_bir_lowering=False, debug=False)

    xq = nc.dram_tensor("xq", [8, 128, 8, 512], BF16, kind="ExternalInput")
    xk = nc.dram_tensor("xk", [8, 128, 8, 512], BF16, kind="ExternalInput")
    xv = nc.dram_tensor("xv", [8, 128, 8, 512], BF16, kind="ExternalInput")
    wq = nc.dram_tensor("wq", [DM, 128], BF16, kind="ExternalInput")
    wk = nc.dram_tensor("wk", [DM, 128], BF16, kind="ExternalInput")
    wv = nc.dram_tensor("wv", [DM, 128], BF16, kind="ExternalInput")
    wo = nc.dram_tensor("wo", [128, DM], BF16, kind="ExternalInput")
    bqk = nc.dram_tensor("bqk", [128, 2], F32, kind="ExternalInput")
    bv = nc.dram_tensor("bv", [128, 1], F32, kind="ExternalInput")
    outT = nc.dram_tensor("outT", [8, 128, 8, 512], BF16, kind="ExternalOutput")

    Exp = mybir.ActivationFunctionType.Exp

    with tile.TileContext(nc) as tc:
        with (
            tc.tile_pool(name="const", bufs=1) as const,
            tc.tile_pool(name="big", bufs=1) as big,
            tc.tile_pool(name="xpool", bufs=1) as xpool,
            tc.tile_pool(name="attn", bufs=AV_LAG + 2) as attnp,
            tc.tile_pool(name="norm", bufs=2) as normp,
            tc.tile_pool(name="osb", bufs=2) as osb,
            tc.tile_pool(name="vtpool", bufs=3) as vtpool,
            tc.tile_pool(name="mm_ps", bufs=2, space="PSUM") as mm_ps,
            tc.tile_pool(name="sc_ps", bufs=2, space="PSUM") as sc_ps,
            tc.tile_pool(name="y_ps", bufs=2, space="PSUM") as y_ps,
        ):
            # ---- weights / biases ----
            wk_sb = const.tile([128, 8, 128], BF16, tag="wk")
            nc.sync.dma_start(wk_sb[:], wk.ap().rearrange("(c p) d -> p c d", p=128))
            wq_sb = const.tile([128, 8, 128], BF16, tag="wq")
            nc.sync.dma_start(wq_sb[:], wq.ap().rearrange("(c p) d -> p c d", p=128))
            wv_sb = const.tile([128, 8, 128], BF16, tag="wv")
            nc.sync.dma_start(wv_sb[:], wv.ap().rearrange("(c p) d -> p c d", p=128))
            wo_sb = const.tile([128, DM], BF16, tag="wo")
            nc.sync.dma_start(wo_sb[:], wo[:, :])
            bqk_sb = const.tile([128, 2], F32, tag="bqk")
            nc.sync.dma_start(bqk_sb[:], bqk[:, :])
            bv_sb = const.tile([128, 1], F32, tag="bv")
            nc.sync.dma_start(bv_sb[:], bv[:, :])
            # per-head v bias at partitions 0:64 (col h = head h)
            bv2_sb = const.tile([64, 2], F32, tag="bv2")
            nc.sync.dma_start(
                bv2_sb[:], bv.ap().rearrange("(h p) c -> p (h c)", h=2))
            # two stacked 64x64 identities so transposes of vT slices at
            # partition offsets 0 and 64 both have a matching-base identity
            ident = const.tile([128, 64], BF16, tag="ident")
            make_identity(nc, ident[0:64, :])
            nc.sync.dma_start(ident[64:128, :], ident[0:64, :])

            # ---- persistent activations ----
            qT = big.tile([128, TOK], BF16, tag="qT")     # [hd, tok]
            kT = big.tile([128, TOK], BF16, tag="kT")     # [hd, tok]
            # v blocks: 32 token-blocks of [128 tok, 2*(64+1)]; col 64 of each
            # per-head group is the ones column (softmax denominator trick)
            vA = big.tile([128, 32 * 130], BF16, tag="vA")
            yT = big.tile([128, TOK], BF16, tag="yT")     # attn out [hd, tok]

            nc.vector.memset(
                vA[:].rearrange("p (b h c) -> p b h c", h=2, c=65)[:, :, :, 64:65], 1.0
            )

            prefetched = {}

            def prefetch(key, tb):
                dram = {"q": xq, "k": xk, "v": xv}[key]
                # per-slot tag (bufs=1): batch-1's (key, tb+4) reuses exactly
                # the buffer of (key, tb), with a WAR dep on its reads
                t = xpool.tile([128, 8, 512], BF16,
                               tag=f"x{key}{tb % 4}", name="xt")
                nc.sync.dma_start(t[:], dram[tb])
                prefetched[(key, tb)] = t

            def proj_qk_steps(tb, which):
                """One qk projection as 8 single-MM closures (last one evicts)."""
                key, w_sb, dstT, bcol = (
                    ("q", wq_sb, qT, 0),
                    ("k", wk_sb, kT, 1),
                )[which]
                state = {}

                def step(mc):
                    if mc == 0:
                        state["xt"] = prefetched.pop((key, tb))
                        state["ps"] = mm_ps.tile([128, 512], F32, tag="mm", name="ps")
                    nc.tensor.matmul(
                        state["ps"][:], lhsT=w_sb[:, mc, :],
                        rhs=state["xt"][:, mc, :],
                        start=(mc == 0), stop=(mc == 7),
                    )
                    if mc == 7:
                        nc.vector.tensor_scalar_add(
                            dstT[:, tb * 512:(tb + 1) * 512], state["ps"][:],
                            bqk_sb[:, bcol:bcol + 1],
                        )
                        if tb < 4:
                            # batch-1 prefetch reuses this tile's buffer; it
                            # must be EMITTED after the last read of the old
                            # tile or the DMA races the projection
                            prefetch(key, tb + 4)
                return [(lambda mc=mc: step(mc), 1) for mc in range(8)]

            def proj_v_steps(tb):
                """One v projection: 8 single-MM closures, 2 per-head bias
                evicts, then 8 transpose+copy closures filling vA (DMA-xbar
                transpose into a zero-offset temp when DMA_VT)."""
                state = {}

                def step(mc):
                    if mc == 0:
                        state["xt"] = prefetched.pop(("v", tb))
                        state["ps"] = mm_ps.tile([128, 512], F32, tag="mm", name="ps")
                    nc.tensor.matmul(
                        state["ps"][:], lhsT=wv_sb[:, mc, :],
                        rhs=state["xt"][:, mc, :],
                        start=(mc == 0), stop=(mc == 7),
                    )
                    if mc == 7 and not DMA_VT:
                        state["vt"] = vtpool.tile(
                            [128, 512], BF16, tag="vt", name="vt")
                        nc.vector.tensor_scalar_add(
                            state["vt"][:], state["ps"][:], bv_sb[:, 0:1])
                    if mc == 7 and tb < 4:
                        prefetch("v", tb + 4)

                def evict(hl):
                    # per-head partition-0 tiles (xbar src must start at p0)
                    t = vtpool.tile([64, 512], BF16, tag=f"vt{hl}", name="vt")
                    nc.vector.tensor_scalar_add(
                        t[:], state["ps"][hl * 64:(hl + 1) * 64, :],
                        bv2_sb[:, hl:hl + 1])
                    state[f"vt{hl}"] = t

                def tstep(j, hl):
                    base = (tb * 4 + j) * 130
                    if DMA_VT:
                        tp = vtpool.tile([128, 64], BF16, tag="tp", name="tp")
                        nc.sync.dma_start(
                            tp[:], state[f"vt{hl}"][:, j * 128:(j + 1) * 128],
                            transpose=True)
                        # gpsimd (idle, off the norm critical path) scatters
                        # into the 65-stride vA layout
                        nc.gpsimd.tensor_copy(
                            vA[:, base + hl * 65: base + hl * 65 + 64], tp[:])
                    else:
                        tp = mm_ps.tile([128, 64], BF16, tag="mm", name="tp")
                        nc.tensor.transpose(
                            tp[:],
                            state["vt"][hl * 64:(hl + 1) * 64,
                                        j * 128:(j + 1) * 128],
                            ident[hl * 64:(hl + 1) * 64, :],
                        )
                        nc.vector.tensor_copy(
                            vA[:, base + hl * 65: base + hl * 65 + 64], tp[:])

                mm_steps = [(lambda mc=mc: step(mc), 1) for mc in range(8)]
                if DMA_VT:
                    return (mm_steps
                            + [(lambda hl=hl: evict(hl), 0) for hl in range(2)]
                            + [(lambda j=j, hl=hl: tstep(j, hl), 0)
                               for j in range(4) for hl in range(2)])
                return (mm_steps
                        + [(lambda j=j, hl=hl: tstep(j, hl), 1)
                           for j in range(4) for hl in range(2)])

            def out_proj_steps(n, qtr):
                """One 512-token output-projection stripe: 8 (MM + bf16 copy)
                closures; the last also DMAs the stripe out."""
                tb = n * 4 + qtr
                state = {}

                def step(mb):
                    if mb == 0:
                        state["o"] = osb.tile(
                            [128, 8, 512], BF16, tag="o", name="o_sb")
                    ps = mm_ps.tile([128, 512], F32, tag="mm", name="ps")
                    nc.tensor.matmul(
                        ps[:], lhsT=wo_sb[:, mb * 128:(mb + 1) * 128],
                        rhs=yT[:, tb * 512:(tb + 1) * 512],
                        start=True, stop=True,
                    )
                    nc.vector.tensor_copy(state["o"][:, mb, :], ps[:])
                    if mb == 7:
                        nc.sync.dma_start(outT[tb], state["o"][:])
                return [(lambda mb=mb: step(mb), 1) for mb in range(8)]

            fill = FillSched()

            def norm_qtr(n, qtr, yaccs):
                kq0 = n * SEQ + qtr * 512
                for h in range(2):
                    hp = h * 64
                    # single copy releases the yacc PSUM buffer fast; the
                    # rest of the chain runs off the PE critical path.
                    # (gpsimd can't access PSUM; custom DVE ops can't either)
                    ycp = normp.tile([65, 512], F32, tag="ycp", name="ycp")
                    nc.vector.tensor_copy(ycp[:], yaccs[h][:])
                    # custom DVE op needs a partition-0 input
                    dsb = normp.tile([1, 512], F32, tag="dsb", name="dsb")
                    nc.vector.tensor_copy(dsb[:], ycp[64:65, :])
                    recip = normp.tile([1, 512], F32, tag="recip", name="recip")
                    nc.vector.reciprocal_approx_fast(recip[:], dsb[:])
                    bcast = normp.tile([64, 512], F32, tag="bcast", name="bcast")
                    nc.gpsimd.partition_broadcast(bcast[:], recip[:])
                    nc.vector.tensor_mul(
                        yT[hp:hp + 64, kq0:kq0 + 512],
                        ycp[0:64, :], bcast[:],
                    )
                fill.add_front(f"out{n}{qtr}", out_proj_steps(n, qtr))

            def do_av(at, n, qtr, lb, yaccs):
                lt = n * 16 + lb
                fill.require(f"v{n * 4 + lb // 4}")
                for h in range(2):
                    nc.tensor.matmul(
                        yaccs[h][:],
                        lhsT=vA[:, lt * 130 + h * 65: lt * 130 + h * 65 + 65],
                        rhs=at[:, h * 512:(h + 1) * 512],
                        start=(lb == 0), stop=(lb == 15),
                    )
                if lb == 15:
                    norm_qtr(n, qtr, yaccs)

            # ---- prefetch: batch 0, k0/q0 in small chunks first so the
            # first projection matmuls start as soon as 128KB lands ----
            def prefetch_chunked(key, tb):
                dram = {"q": xq, "k": xk, "v": xv}[key]
                t = xpool.tile([128, 8, 512], BF16,
                               tag=f"x{key}{tb % 4}", name="xt")
                for mc in range(8):
                    nc.sync.dma_start(t[:, mc, :], dram[tb][:, mc, :])
                prefetched[(key, tb)] = t

            prefetch_chunked("k", 0)
            prefetch_chunked("q", 0)
            for tb in (1, 2, 3):
                prefetch("k", tb)
            for tb in range(4):
                prefetch("v", tb)
            for tb in (1, 2, 3):
                prefetch("q", tb)

            # ---- upfront: k0 + q0 projections only ----
            for f, _ in proj_qk_steps(0, 1):
                f()
            for f, _ in proj_qk_steps(0, 0):
                f()
            fill.done.update({"k0", "q0"})

            # deadline order: scores(lb) need k-tb(lb//4) / q-tb(qtr);
            # AV (lag 12) needs v-tb((lb-12)//4); batch 1 follows
            # (batch-1 prefetches are emitted by the consumption hooks
            # inside proj_*_steps — buffer-exact, race-free)
            units = [
                ("k1", proj_qk_steps(1, 1)), ("k2", proj_qk_steps(2, 1)),
                ("v0", proj_v_steps(0)), ("k3", proj_qk_steps(3, 1)),
                ("v1", proj_v_steps(1)), ("q1", proj_qk_steps(1, 0)),
                ("v2", proj_v_steps(2)), ("v3", proj_v_steps(3)),
                ("q2", proj_qk_steps(2, 0)), ("q3", proj_qk_steps(3, 0)),
                ("k4", proj_qk_steps(4, 1)), ("q4", proj_qk_steps(4, 0)),
                ("k5", proj_qk_steps(5, 1)), ("k6", proj_qk_steps(6, 1)),
                ("v4", proj_v_steps(4)), ("k7", proj_qk_steps(7, 1)),
                ("v5", proj_v_steps(5)), ("q5", proj_qk_steps(5, 0)),
                ("v6", proj_v_steps(6)), ("v7", proj_v_steps(7)),
                ("q6", proj_qk_steps(6, 0)), ("q7", proj_qk_steps(7, 0)),
            ]
            for name, unit in units:
                fill.add(name, unit)

            # ---- the continuous attention stream ----
            pend = deque()
            qtr_yaccs = {}
            NIT = NB * 64
            for i in range(NIT):
                n, r = divmod(i, 64)
                qtr, lb = divmod(r, 16)
                lt = n * 16 + lb
                kq0 = n * SEQ + qtr * 512
                if lb == 0:
                    qtr_yaccs[(n, qtr)] = [
                        y_ps.tile([65, 512], F32, tag="yacc", name="yacc")
                        for _ in range(2)
                    ]
                fill.require(f"k{n * 4 + lb // 4}")
                fill.require(f"q{n * 4 + qtr}")
                sp = sc_ps.tile([128, 1024], F32, tag="sc", name="sp")
                for h in range(2):
                    # K=64, base partitions 0/64 -> concurrent row-tiles
                    nc.tensor.matmul(
                        sp[:, h * 512:(h + 1) * 512],
                        lhsT=kT[h * 64:(h + 1) * 64, lt * 128:(lt + 1) * 128],
                        rhs=qT[h * 64:(h + 1) * 64, kq0:kq0 + 512],
                        start=True, stop=True,
                    )
                at = attnp.tile([128, 1024], BF16, tag="at", name="at")
                nc.scalar.activation(at[:], sp[:], Exp, scale=0.125)
                pend.append((at, n, qtr, lb, qtr_yaccs[(n, qtr)]))
                limit = AV_LAG if i < NIT - (AV_LAG - AV_LAG_TAIL) else AV_LAG_TAIL
                while len(pend) > limit:
                    args = pend.popleft()
                    do_av(*args)
                    if args[3] == 15:
                        del qtr_yaccs[(args[1], args[2])]
                fill.pop_steps(4 if i < 24 else (3 if i < 48 else 2))
            for args in pend:
                do_av(*args)
            fill.drain()

    nc.compile()
    return nc


def get_program():
    if "nc" not in _prog_cache:
        _prog_cache["nc"] = build_program()
    return _prog_cache["nc"]


def _tile_x(x):
    # [TOK, DM] f32 -> bf16 tiles [8 tb, 128 p, 8 mc, 512 f]:
    # t[tb,p,mc,f] = x[tb*512+f, mc*128+p]
    t = x.reshape(8, 512, 8, 128).astype(BFNP)   # [tb, f, mc, p]
    return np.ascontiguousarray(np.transpose(t, (0, 3, 2, 1)))


def make_in_maps(query, key, value, Wq, bq, Wk, bk, Wv, bv, Wo):
    """Host-side shard + layout. Returns list of 8 per-core input dicts."""
    xq = _tile_x(query.reshape(TOK, DM))
    xk = _tile_x(key.reshape(TOK, DM))
    xv = _tile_x(value.reshape(TOK, DM))

    in_maps = []
    for c in range(N_CORES):
        h0 = HC * c
        # W[h,d,m] slice -> [m, hl*64+d]
        wq_c = np.ascontiguousarray(
            np.transpose(Wq[h0:h0 + HC], (2, 0, 1)).reshape(DM, 128)).astype(BFNP)
        wk_c = np.ascontiguousarray(
            np.transpose(Wk[h0:h0 + HC], (2, 0, 1)).reshape(DM, 128)).astype(BFNP)
        wv_c = np.ascontiguousarray(
            np.transpose(Wv[h0:h0 + HC], (2, 0, 1)).reshape(DM, 128)).astype(BFNP)
        wo_c = np.ascontiguousarray(
            Wo[:, 128 * c:128 * (c + 1)].T).astype(BFNP)
        bqk_c = np.stack(
            [bq[h0:h0 + HC].reshape(128), bk[h0:h0 + HC].reshape(128)], axis=1
        ).astype(np.float32)
        bv_c = bv[h0:h0 + HC].reshape(128, 1).astype(np.float32)
        in_maps.append({
            "xq": xq, "xk": xk, "xv": xv,
            "wq": wq_c, "wk": wk_c, "wv": wv_c, "wo": wo_c,
            "bqk": bqk_c, "bv": bv_c,
        })
    return in_maps


def untile_out(res_list):
    """Sum per-core tiled bf16 partials in f32 -> [DM, TOK] f32."""
    acc = np.zeros((8, 128, 8, 512), np.float32)
    for r in res_list:
        acc += r["outT"].astype(np.float32)
    # [tb, p, mb, f] -> [mb*128+p, tb*512+f]
    return np.ascontiguousarray(np.transpose(acc, (2, 1, 0, 3))).reshape(DM, TOK)


def kernel(query, key, value, Wq, bq, Wk, bk, Wv, bv, Wo, bo):
    nc = get_program()
    in_maps = make_in_maps(query, key, value, Wq, bq, Wk, bk, Wv, bv, Wo)
    res = run_bass_kernel_spmd(nc, in_maps, list(range(N_CORES)))
    out_t = untile_out(res.results)
    out = out_t.T.reshape(NB, SEQ, DM) + bo.astype(np.float32)
    return out


# revision 29
# speedup vs baseline: 1.3797x; 1.0022x over previous
"""Multi-head attention (N=2, K=2048, M=1024, H=16, D=64) on 8 TRN2 cores.

Sharding: tensor-parallel over heads — core c owns heads (2c, 2c+1).
Each core computes q/k/v projections for its 2 heads (full sequence),
attention, and a rank-128 partial of the output projection (its 128 rows
of Wo's input dim). Host sums the 8 partials and adds bo. No device
collectives.

On-device layouts (per core):
  xq/xk/xv [8 tb, 128 p, 8 mc, 512 f] bf16  host-tiled transposed inputs:
           [tb, p, mc, f] = x[tok=tb*512+f, m=mc*128+p], tok = n*2048+k
           -> one contiguous 1MB DMA per (tensor, tb)
  wq/wk/wv [1024 m, 128 hd] bf16   W[h,d,m] -> [m, hl*64+d] for local heads
  wo       [128 hd, 1024 mo] bf16  Wo[:, c*128:(c+1)*128].T
  bqk      [128, 2] f32, bv [128, 1] f32
  outT     [8 tb, 128 p, 8 mb, 512 f] bf16  tiled partial (1MB DMA per tb)

Attention is ONE continuous software pipeline over 128 iterations
(2 batches x 4 kq-quarters x 16 l-blocks) with no barriers: per
iteration the two heads' K=64 score matmuls run CONCURRENTLY on
disjoint PE row-groups (base partitions 0/64 -> tile_position row
packing), one [128,1024] ACTIVATE exps both heads, and AV matmuls
(ones-column trick, M=65) lag 12 iterations behind. Quarter
normalization (reciprocal_approx_fast + gpsimd partition_broadcast +
PSUM-direct multiply) and output-projection stripes are emitted
mid-stream, so ScalarE (the exp wall, ~143us) never starves at
quarter/batch boundaries. Projections are single-matmul fill steps
paced 2-4 per iteration into the PE's slack under ScalarE; a
deadline `require()` forces emission of any unit a consumer needs
(emission order defines Tile dependencies). Output partials are cast
to bf16 (halves the out-DMA); the host sums partials in f32.
"""
from collections import deque

import numpy as np
import ml_dtypes

import concourse.bass as bass
import concourse.tile as tile
from concourse.masks import make_identity
from concourse import bacc, mybir
from concourse.bass_utils import run_bass_kernel_spmd

F32 = mybir.dt.float32
BF16 = mybir.dt.bfloat16
BFNP = ml_dtypes.bfloat16

N_CORES = 8
DM = 1024          # d_model
TOK = 4096         # N*K tokens
SEQ = 2048         # tokens per batch
NB = 2             # batches
HC = 2             # heads per core
D = 64             # head dim

AV_LAG = 12        # iterations AV trails scores/exp
AV_LAG_TAIL = 3    # shrink lag near the end to cut the drain tail
DMA_VT = False     # v-transposes via DMA xbar instead of PE+DVE

_prog_cache = {}


class FillSched:
    """Named-unit fill scheduler. Units are atomic (they share the mm_ps
    pool and must not interleave with each other); steps within the
    active unit are paced out by PE cost (matmul steps cost 1, DVE/DMA
    steps cost 0). require(name) forces full emission of every unit up
    to and including `name` — emission order defines Tile dependencies,
    so any unit a consumer reads from MUST be emitted (not just queued)
    before the consumer."""

    def __init__(self):
        self.order = deque()      # (name, deque((fn, cost)))
        self.cur_name = None
        self.cur = deque()
        self.done = set()

    def add(self, name, unit):
        self.order.append((name, deque(unit)))

    def add_front(self, name, unit):
        self.order.appendleft((name, deque(unit)))

    def _finish_cur(self):
        while self.cur:
            self.cur.popleft()[0]()
        if self.cur_name is not None:
            self.done.add(self.cur_name)
            self.cur_name = None

    def pop_steps(self, budget):
        while budget > 0:
            if not self.cur:
                if self.cur_name is not None:
                    self.done.add(self.cur_name)
                    self.cur_name = None
                if not self.order:
                    return
                self.cur_name, self.cur = self.order.popleft()
            fn, cost = self.cur.popleft()
            fn()
            budget -= cost
        if not self.cur and self.cur_name is not None:
            self.done.add(self.cur_name)
            self.cur_name = None

    def require(self, name):
        if name in self.done:
            return
        if self.cur_name is not None:
            if self.cur_name == name:
                self._finish_cur()
                return
            self._finish_cur()
        while name not in self.done:
            assert self.order, f"unit {name} was never queued"
            self.cur_name, self.cur = self.order.popleft()
            self._finish_cur()

    def drain(self):
        self._finish_cur()
        while self.order:
            self.cur_name, self.cur = self.order.popleft()
            self._finish_cur()


def build_program():
    nc = bacc.Bacc("TRN2", target_bir_lowering=False, debug=False)

    xq = nc.dram_tensor("xq", [8, 128, 8, 512], BF16, kind="ExternalInput")
    xk = nc.dram_tensor("xk", [8, 128, 8, 512], BF16, kind="ExternalInput")
    xv = nc.dram_tensor("xv", [8, 128, 8, 512], BF16, kind="ExternalInput")
    wq = nc.dram_tensor("wq", [DM, 128], BF16, kind="ExternalInput")
    wk = nc.dram_tensor("wk", [DM, 128], BF16, kind="ExternalInput")
    wv = nc.dram_tensor("wv", [DM, 128], BF16, kind="ExternalInput")
    wo = nc.dram_tensor("wo", [128, DM], BF16, kind="ExternalInput")
    bqk = nc.dram_tensor("bqk", [128, 2], F32, kind="ExternalInput")
    bv = nc.dram_tensor("bv", [128, 1], F32, kind="ExternalInput")
    outT = nc.dram_tensor("outT", [8, 128, 8, 512], BF16, kind="ExternalOutput")

    Exp = mybir.ActivationFunctionType.Exp

    with tile.TileContext(nc) as tc:
        with (
            tc.tile_pool(name="const", bufs=1) as const,
            tc.tile_pool(name="big", bufs=1) as big,
            tc.tile_pool(name="xpool", bufs=1) as xpool,
            tc.tile_pool(name="attn", bufs=AV_LAG + 2) as attnp,
            tc.tile_pool(name="norm", bufs=2) as normp,
            tc.tile_pool(name="osb", bufs=2) as osb,
            tc.tile_pool(name="vtpool", bufs=3) as vtpool,
            tc.tile_pool(name="mm_ps", bufs=2, space="PSUM") as mm_ps,
            tc.tile_pool(name="sc_ps", bufs=2, space="PSUM") as sc_ps,
            tc.tile_pool(name="y_ps", bufs=2, space="PSUM") as y_ps,
        ):
            # ---- weights / biases ----
            wk_sb = const.tile([128, 8, 128], BF16, tag="wk")
            nc.sync.dma_start(wk_sb[:], wk.ap().rearrange("(c p) d -> p c d", p=128))
            wq_sb = const.tile([128, 8, 128], BF16, tag="wq")
            nc.sync.dma_start(wq_sb[:], wq.ap().rearrange("(c p) d -> p c d", p=128))
            wv_sb = const.tile([128, 8, 128], BF16, tag="wv")
            nc.sync.dma_start(wv_sb[:], wv.ap().rearrange("(c p) d -> p c d", p=128))
            wo_sb = const.tile([128, DM], BF16, tag="wo")
            nc.sync.dma_start(wo_sb[:], wo[:, :])
            bqk_sb = const.tile([128, 2], F32, tag="bqk")
            nc.sync.dma_start(bqk_sb[:], bqk[:, :])
            bv_sb = const.tile([128, 1], F32, tag="bv")
            nc.sync.dma_start(bv_sb[:], bv[:, :])
            # per-head v bias at partitions 0:64 (col h = head h)
            bv2_sb = const.tile([64, 2], F32, tag="bv2")
            nc.sync.dma_start(
                bv2_sb[:], bv.ap().rearrange("(h p) c -> p (h c)", h=2))
            # two stacked 64x64 identities so transposes of vT slices at
            # partition offsets 0 and 64 both have a matching-base identity
            ident = const.tile([128, 64], BF16, tag="ident")
            make_identity(nc, ident[0:64, :])
            nc.sync.dma_start(ident[64:128, :], ident[0:64, :])

            # ---- persistent activations ----
            qT = big.tile([128, TOK], BF16, tag="qT")     # [hd, tok]
            kT = big.tile([128, TOK], BF16, tag="kT")     # [hd, tok]
            # v blocks: 32 token-blocks of [128 tok, 2*(64+1)]; col 64 of each
            # per-head group is the ones column (softmax denominator trick)
            vA = big.tile([128, 32 * 130], BF16, tag="vA")
            yT = big.tile([128, TOK], BF16, tag="yT")     # attn out [hd, tok]

            nc.vector.memset(
                vA[:].rearrange("p (b h c) -> p b h c", h=2, c=65)[:, :, :, 64:65], 1.0
            )

            prefetched = {}

            def prefetch(key, tb):
                dram = {"q": xq, "k": xk, "v": xv}[key]
                # per-slot tag (bufs=1): batch-1's (key, tb+4) reuses exactly
                # the buffer of (key, tb), with a WAR dep on its reads
                t = xpool.tile([128, 8, 512], BF16,
                               tag=f"x{key}{tb % 4}", name="xt")
                nc.sync.dma_start(t[:], dram[tb])
                prefetched[(key, tb)] = t

            def proj_qk_steps(tb, which):
                """One qk projection as 8 single-MM closures (last one evicts)."""
                key, w_sb, dstT, bcol = (
                    ("q", wq_sb, qT, 0),
                    ("k", wk_sb, kT, 1),
                )[which]
                state = {}

                def step(mc):
                    if mc == 0:
                        state["xt"] = prefetched.pop((key, tb))
                        state["ps"] = mm_ps.tile([128, 512], F32, tag="mm", name="ps")
                    nc.tensor.matmul(
                        state["ps"][:], lhsT=w_sb[:, mc, :],
                        rhs=state["xt"][:, mc, :],
                        start=(mc == 0), stop=(mc == 7),
                    )
                    if mc == 7:
                        nc.vector.tensor_scalar_add(
                            dstT[:, tb * 512:(tb + 1) * 512], state["ps"][:],
                            bqk_sb[:, bcol:bcol + 1],
                        )
                        if tb < 4:
                            # batch-1 prefetch reuses this tile's buffer; it
                            # must be EMITTED after the last read of the old
                            # tile or the DMA races the projection
                            prefetch(key, tb + 4)
                return [(lambda mc=mc: step(mc), 1) for mc in range(8)]

            def proj_v_steps(tb):
                """One v projection: 8 single-MM closures, 2 per-head bias
                evicts, then 8 transpose+copy closures filling vA (DMA-xbar
                transpose into a zero-offset temp when DMA_VT)."""
                state = {}

                def step(mc):
                    if mc == 0:
                        state["xt"] = prefetched.pop(("v", tb))
                        state["ps"] = mm_ps.tile([128, 512], F32, tag="mm", name="ps")
                    nc.tensor.matmul(
                        state["ps"][:], lhsT=wv_sb[:, mc, :],
                        rhs=state["xt"][:, mc, :],
                        start=(mc == 0), stop=(mc == 7),
                    )
                    if mc == 7 and not DMA_VT:
                        state["vt"] = vtpool.tile(
                            [128, 512], BF16, tag="vt", name="vt")
                        nc.vector.tensor_scalar_add(
                            state["vt"][:], state["ps"][:], bv_sb[:, 0:1])
                    if mc == 7 and tb < 4:
                        prefetch("v", tb + 4)

                def evict(hl):
                    # per-head partition-0 tiles (xbar src must start at p0)
                    t = vtpool.tile([64, 512], BF16, tag=f"vt{hl}", name="vt")
                    nc.vector.tensor_scalar_add(
                        t[:], state["ps"][hl * 64:(hl + 1) * 64, :],
                        bv2_sb[:, hl:hl + 1])
                    state[f"vt{hl}"] = t

                def tstep(j, hl):
                    base = (tb * 4 + j) * 130
                    if DMA_VT:
                        tp = vtpool.tile([128, 64], BF16, tag="tp", name="tp")
                        nc.sync.dma_start(
                            tp[:], state[f"vt{hl}"][:, j * 128:(j + 1) * 128],
                            transpose=True)
                        # gpsimd (idle, off the norm critical path) scatters
                        # into the 65-stride vA layout
                        nc.gpsimd.tensor_copy(
                            vA[:, base + hl * 65: base + hl * 65 + 64], tp[:])
                    else:
                        tp = mm_ps.tile([128, 64], BF16, tag="mm", name="tp")
                        nc.tensor.transpose(
                            tp[:],
                            state["vt"][hl * 64:(hl + 1) * 64,
                                        j * 128:(j + 1) * 128],
                            ident[hl * 64:(hl + 1) * 64, :],
                        )
                        nc.vector.tensor_copy(
                            vA[:, base + hl * 65: base + hl * 65 + 64], tp[:])

                mm_steps = [(lambda mc=mc: step(mc), 1) for mc in range(8)]
                if DMA_VT:
                    return (mm_steps
                            + [(lambda hl=hl: evict(hl), 0) for hl in range(2)]
                            + [(lambda j=j, hl=hl: tstep(j, hl), 0)
                               for j in range(4) for hl in range(2)])
                return (mm_steps
                        + [(lambda j=j, hl=hl: tstep(j, hl), 1)
                           for j in range(4) for hl in range(2)])

            def out_proj_steps(n, qtr, tail=False):
                """One 512-token output-projection stripe: 8 (MM + bf16 copy)
                closures; the last also DMAs the stripe out. In tail mode the
                exp stream is over, so ScalarE helps with the casts and the
                DMA is split to overlap them."""
                tb = n * 4 + qtr
                state = {}

                def step(mb):
                    if mb == 0:
                        state["o"] = osb.tile(
                            [128, 8, 512], BF16, tag="o", name="o_sb")
                    ps = mm_ps.tile([128, 512], F32, tag="mm", name="ps")
                    nc.tensor.matmul(
                        ps[:], lhsT=wo_sb[:, mb * 128:(mb + 1) * 128],
                        rhs=yT[:, tb * 512:(tb + 1) * 512],
                        start=True, stop=True,
                    )
                    if tail and mb % 2 == 0:
                        nc.scalar.copy(state["o"][:, mb, :], ps[:])
                    else:
                        nc.vector.tensor_copy(state["o"][:, mb, :], ps[:])
                    if tail and mb == 3:
                        nc.sync.dma_start(
                            outT[tb, :, 0:4, :], state["o"][:, 0:4, :])
                    elif tail and mb == 7:
                        nc.sync.dma_start(
                            outT[tb, :, 4:8, :], state["o"][:, 4:8, :])
                    elif mb == 7:
                        nc.sync.dma_start(outT[tb], state["o"][:])
                return [(lambda mb=mb: step(mb), 1) for mb in range(8)]

            fill = FillSched()

            def norm_qtr(n, qtr, yaccs):
                kq0 = n * SEQ + qtr * 512
                for h in range(2):
                    hp = h * 64
                    # single copy releases the yacc PSUM buffer fast; the
                    # rest of the chain runs off the PE critical path.
                    # (gpsimd can't access PSUM; custom DVE ops can't either)
                    ycp = normp.tile([65, 512], F32, tag="ycp", name="ycp")
                    nc.vector.tensor_copy(ycp[:], yaccs[h][:])
                    # custom DVE op needs a partition-0 input
                    dsb = normp.tile([1, 512], F32, tag="dsb", name="dsb")
                    nc.vector.tensor_copy(dsb[:], ycp[64:65, :])
                    recip = normp.tile([1, 512], F32, tag="recip", name="recip")
                    nc.vector.reciprocal_approx_fast(recip[:], dsb[:])
                    bcast = normp.tile([64, 512], F32, tag="bcast", name="bcast")
                    nc.gpsimd.partition_broadcast(bcast[:], recip[:])
                    nc.vector.tensor_mul(
                        yT[hp:hp + 64, kq0:kq0 + 512],
                        ycp[0:64, :], bcast[:],
                    )
                tail = (n == NB - 1 and qtr == 3)
                fill.add_front(f"out{n}{qtr}", out_proj_steps(n, qtr, tail))

            def do_av(at, n, qtr, lb, yaccs):
                lt = n * 16 + lb
                fill.require(f"v{n * 4 + lb // 4}")
                for h in range(2):
                    nc.tensor.matmul(
                        yaccs[h][:],
                        lhsT=vA[:, lt * 130 + h * 65: lt * 130 + h * 65 + 65],
                        rhs=at[:, h * 512:(h + 1) * 512],
                        start=(lb == 0), stop=(lb == 15),
                    )
                if lb == 15:
                    norm_qtr(n, qtr, yaccs)

            # ---- prefetch: batch 0, k0/q0 in small chunks first so the
            # first projection matmuls start as soon as 128KB lands ----
            def prefetch_chunked(key, tb):
                dram = {"q": xq, "k": xk, "v": xv}[key]
                t = xpool.tile([128, 8, 512], BF16,
                               tag=f"x{key}{tb % 4}", name="xt")
                for mc in range(8):
                    nc.sync.dma_start(t[:, mc, :], dram[tb][:, mc, :])
                prefetched[(key, tb)] = t

            prefetch_chunked("k", 0)
            prefetch_chunked("q", 0)
            for tb in (1, 2, 3):
                prefetch("k", tb)
            for tb in range(4):
                prefetch("v", tb)
            for tb in (1, 2, 3):
                prefetch("q", tb)

            # ---- upfront: k0 + q0 projections only ----
            for f, _ in proj_qk_steps(0, 1):
                f()
            for f, _ in proj_qk_steps(0, 0):
                f()
            fill.done.update({"k0", "q0"})

            # deadline order: scores(lb) need k-tb(lb//4) / q-tb(qtr);
            # AV (lag 12) needs v-tb((lb-12)//4); batch 1 follows
            # (batch-1 prefetches are emitted by the consumption hooks
            # inside proj_*_steps — buffer-exact, race-free)
            units = [
                ("k1", proj_qk_steps(1, 1)), ("k2", proj_qk_steps(2, 1)),
                ("v0", proj_v_steps(0)), ("k3", proj_qk_steps(3, 1)),
                ("v1", proj_v_steps(1)), ("q1", proj_qk_steps(1, 0)),
                ("v2", proj_v_steps(2)), ("v3", proj_v_steps(3)),
                ("q2", proj_qk_steps(2, 0)), ("q3", proj_qk_steps(3, 0)),
                ("k4", proj_qk_steps(4, 1)), ("q4", proj_qk_steps(4, 0)),
                ("k5", proj_qk_steps(5, 1)), ("k6", proj_qk_steps(6, 1)),
                ("v4", proj_v_steps(4)), ("k7", proj_qk_steps(7, 1)),
                ("v5", proj_v_steps(5)), ("q5", proj_qk_steps(5, 0)),
                ("v6", proj_v_steps(6)), ("v7", proj_v_steps(7)),
                ("q6", proj_qk_steps(6, 0)), ("q7", proj_qk_steps(7, 0)),
            ]
            for name, unit in units:
                fill.add(name, unit)

            # ---- the continuous attention stream ----
            pend = deque()
            qtr_yaccs = {}
            NIT = NB * 64
            for i in range(NIT):
                n, r = divmod(i, 64)
                qtr, lb = divmod(r, 16)
                lt = n * 16 + lb
                kq0 = n * SEQ + qtr * 512
                if lb == 0:
                    qtr_yaccs[(n, qtr)] = [
                        y_ps.tile([65, 512], F32, tag="yacc", name="yacc")
                        for _ in range(2)
                    ]
                fill.require(f"k{n * 4 + lb // 4}")
                fill.require(f"q{n * 4 + qtr}")
                sp = sc_ps.tile([128, 1024], F32, tag="sc", name="sp")
                for h in range(2):
                    # K=64, base partitions 0/64 -> concurrent row-tiles
                    nc.tensor.matmul(
                        sp[:, h * 512:(h + 1) * 512],
                        lhsT=kT[h * 64:(h + 1) * 64, lt * 128:(lt + 1) * 128],
                        rhs=qT[h * 64:(h + 1) * 64, kq0:kq0 + 512],
                        start=True, stop=True,
                    )
                at = attnp.tile([128, 1024], BF16, tag="at", name="at")
                nc.scalar.activation(at[:], sp[:], Exp, scale=0.125)
                pend.append((at, n, qtr, lb, qtr_yaccs[(n, qtr)]))
                limit = AV_LAG if i < NIT - (AV_LAG - AV_LAG_TAIL) else AV_LAG_TAIL
                # drain AVs two l-blocks at a time (every other iteration):
                # grouping full-array AV matmuls halves the PE row-config
                # switches against the half-array score pairs
                if i % 2 == 1 or len(pend) > limit + 1:
                    while len(pend) > limit - 1 and pend:
                        args = pend.popleft()
                        do_av(*args)
                        if args[3] == 15:
                            del qtr_yaccs[(args[1], args[2])]
                fill.pop_steps(4 if i < 24 else (3 if i < 48 else 2))
            for args in pend:
                do_av(*args)
            fill.drain()

    nc.compile()
    return nc


def get_program():
    if "nc" not in _prog_cache:
        _prog_cache["nc"] = build_program()
    return _prog_cache["nc"]


def _tile_x(x):
    # [TOK, DM] f32 -> bf16 tiles [8 tb, 128 p, 8 mc, 512 f]:
    # t[tb,p,mc,f] = x[tb*512+f, mc*128+p]
    t = x.reshape(8, 512, 8, 128).astype(BFNP)   # [tb, f, mc, p]
    return np.ascontiguousarray(np.transpose(t, (0, 3, 2, 1)))


def make_in_maps(query, key, value, Wq, bq, Wk, bk, Wv, bv, Wo):
    """Host-side shard + layout. Returns list of 8 per-core input dicts."""
    xq = _tile_x(query.reshape(TOK, DM))
    xk = _tile_x(key.reshape(TOK, DM))
    xv = _tile_x(value.reshape(TOK, DM))

    in_maps = []
    for c in range(N_CORES):
        h0 = HC * c
        # W[h,d,m] slice -> [m, hl*64+d]
        wq_c = np.ascontiguousarray(
            np.transpose(Wq[h0:h0 + HC], (2, 0, 1)).reshape(DM, 128)).astype(BFNP)
        wk_c = np.ascontiguousarray(
            np.transpose(Wk[h0:h0 + HC], (2, 0, 1)).reshape(DM, 128)).astype(BFNP)
        wv_c = np.ascontiguousarray(
            np.transpose(Wv[h0:h0 + HC], (2, 0, 1)).reshape(DM, 128)).astype(BFNP)
        wo_c = np.ascontiguousarray(
            Wo[:, 128 * c:128 * (c + 1)].T).astype(BFNP)
        bqk_c = np.stack(
            [bq[h0:h0 + HC].reshape(128), bk[h0:h0 + HC].reshape(128)], axis=1
        ).astype(np.float32)
        bv_c = bv[h0:h0 + HC].reshape(128, 1).astype(np.float32)
        in_maps.append({
            "xq": xq, "xk": xk, "xv": xv,
            "wq": wq_c, "wk": wk_c, "wv": wv_c, "wo": wo_c,
            "bqk": bqk_c, "bv": bv_c,
        })
    return in_maps


def untile_out(res_list):
    """Sum per-core tiled bf16 partials in f32 -> [DM, TOK] f32."""
    acc = np.zeros((8, 128, 8, 512), np.float32)
    for r in res_list:
        acc += r["outT"].astype(np.float32)
    # [tb, p, mb, f] -> [mb*128+p, tb*512+f]
    return np.ascontiguousarray(np.transpose(acc, (2, 1, 0, 3))).reshape(DM, TOK)


def kernel(query, key, value, Wq, bq, Wk, bk, Wv, bv, Wo, bo):
    nc = get_program()
    in_maps = make_in_maps(query, key, value, Wq, bq, Wk, bk, Wv, bv, Wo)
    res = run_bass_kernel_spmd(nc, in_maps, list(range(N_CORES)))
    out_t = untile_out(res.results)
    out = out_t.T.reshape(NB, SEQ, DM) + bo.astype(np.float32)
    return out
